# revision 2
# baseline (speedup 1.0000x reference)
"""Causal self-attention (RMS-normed QK + RoPE + v-mix) on 8 trn2 cores.

Sharding: tensor-parallel over heads x causal-balanced query split.
  - 12 heads -> 4 groups of 3 heads; group g runs on cores (2g, 2g+1).
  - Within a pair, core parity p owns the 8 query tiles with global tile
    index == p (mod 2) (128 rows each).
  - Each core emits a partial projection y_part for its 1024 query rows;
    the host sums the 4 group partials per row.

v2: all matmul operands bf16 (4x PE throughput vs fp32), k-side rms
scale folded into the exp activation's per-partition scale operand,
column-form sum-of-squares for K via tiny matmuls, rope combine done as
a PE identity-matmul accumulate, multiplicative post-exp masking,
reciprocal_approx_fast instead of the (3.8us/call) DVE reciprocal,
coalesced+ordered DMAs.
"""

import os
import sys

sys.path.insert(0, "/opt/trn_rl_repo")

import numpy as np

import concourse.bass as bass
from concourse import mybir
from concourse.tile import TileContext
from concourse.vector_clock import ScopedClock

F32 = mybir.dt.float32
BF16 = mybir.dt.bfloat16
AF = mybir.ActivationFunctionType

T = 2048
D = 768
NH = 12
HD = 64
HPC = 3  # heads per core
C = HPC * HD  # 192 channels per group
NQ = 1024  # query rows per core
NKT = T // 128  # 16 key tiles
NDT = D // 128  # 6 contraction tiles
EPS = float(np.finfo(np.float32).eps)

TRACE = False
TRACE_DIR = None
_CACHED = {}


def _patch_tile_tail():
    """walrus here rejects >1 sync-wait per instruction; TileContext's tail
    drain stacks one wait per active proc.  Spread them over wait_ge's."""
    if getattr(TileContext, "_tail_patched", False):
        return

    def _drain_and_barrier(self, tick_clock, wait_clock):
        nc = self.nc
        collector = nc.sync.nop()
        wait_clock.add_sem_waits(
            collector.ins, ScopedClock({None: tick_clock.global_clock})
        )
        si = collector.ins.sync_info
        waits = list(si.on_wait) if (si and si.on_wait) else []
        if len(waits) > 1:
            by_num = {h.num: h for h in wait_clock.sems.allocated().values()}
            kept, respawn = [], []
            for w in waits:
                if kept and w.id in by_num and w.wait_mode == "sem-ge-imm":
                    respawn.append(w)
                else:
                    kept.append(w)
            si.on_wait = kept
            for w in respawn:
                nc.sync.wait_ge(by_num[w.id], w.wait_value)
        nc.sync.drain()
        nc.all_engine_barrier()
        assert self.sems is not None
        popped = nc._tile_sem_poison_stack.pop()
        assert popped is self._sem_poison
        nc.clear_and_free_semaphores(list(self.sems.allocated().values()))
        nc.all_engine_barrier()

    TileContext._drain_and_barrier = _drain_and_barrier
    TileContext._tail_patched = True


def _split_multiwait_bir(bir_json):
    """Rewrite serialized BIR so no instruction carries more than one sync
    wait (this walrus build rejects >1): extra waits move onto single-wait
    NoOps inserted just before the instruction on the same engine."""
    import json as _json

    d = _json.loads(bir_json)
    for fn in d["functions"]:
        for bb in fn["blocks"]:
            out = []
            for inst in bb["instructions"]:
                si = inst.get("sync_info") or {}
                waits = si.get("on_wait") or []
                if len(waits) > 1:
                    for wi, w in enumerate(waits[:-1]):
                        out.append(
                            {
                                "name": f"{inst['name']}-wsplit{wi}",
                                "opcode": "EventSemaphore",
                                "engine": inst["engine"],
                                "debug": inst.get("debug", 0),
                                "ins": [],
                                "outs": [],
                                "sync_info": {"on_update": [], "on_wait": [w]},
                            }
                        )
                    si["on_wait"] = [waits[-1]]
                out.append(inst)
            bb["instructions"] = out
    enc = _json.dumps(d)
    return enc.encode() if isinstance(bir_json, bytes) else enc


def _patch_wait_split():
    import concourse.bass_utils as bu
    import concourse.bass2jax as b2j

    if getattr(bu, "_wait_split_patched", False):
        return
    orig = bu.compile_bir_kernel

    def wrapped(bir_json, tmpdir, neff_name="file.neff"):
        return orig(_split_multiwait_bir(bir_json), tmpdir, neff_name=neff_name)

    bu.compile_bir_kernel = wrapped
    b2j.compile_bir_kernel = wrapped
    bu._wait_split_patched = True


def j0_of(k):
    # first compact q-block (0..7) whose global tile can see key tile k,
    # under the uniform bound (odd-parity core's view; even cores get one
    # fully-masked diagonal block per odd k via the data mask)
    return k // 2


def build_nc():
    KPHASE = int(os.environ.get("KPHASE", "3"))
    SKIP = set(os.environ.get("KSKIP", "").split(","))
    _patch_tile_tail()
    _patch_wait_split()
    nc = bass.Bass("TRN2")

    # chunk-major packed inputs (see _host_prep for layouts)
    xtp = nc.dram_tensor("xtp", [128, 4 * NDT * 512], BF16, kind="ExternalInput")
    xqp = nc.dram_tensor("xqp", [128, 2 * NDT * 512], BF16, kind="ExternalInput")
    wq = nc.dram_tensor("wq", [128, NDT * 256], BF16, kind="ExternalInput")
    wk = nc.dram_tensor("wk", [128, NDT * 256], BF16, kind="ExternalInput")
    wv = nc.dram_tensor("wv", [128, NDT * C], BF16, kind="ExternalInput")
    wpn = nc.dram_tensor("wpn", [128, 2 * D], BF16, kind="ExternalInput")
    vin = nc.dram_tensor("vin", [128, NKT * C], BF16, kind="ExternalInput")
    c4k = nc.dram_tensor("c4k", [128, T], BF16, kind="ExternalInput")
    s4k = nc.dram_tensor("s4k", [128, T], BF16, kind="ExternalInput")
    c4q = nc.dram_tensor("c4q", [128, NQ], BF16, kind="ExternalInput")
    s4q = nc.dram_tensor("s4q", [128, NQ], BF16, kind="ExternalInput")
    m01 = nc.dram_tensor("m01", [128, NKT * 128], BF16, kind="ExternalInput")
    vz = nc.dram_tensor("vz", [128, 384], BF16, kind="ExternalInput")
    perm = nc.dram_tensor("perm", [128, 128], BF16, kind="ExternalInput")
    idw = nc.dram_tensor("idw", [128, 128], BF16, kind="ExternalInput")
    qsn = nc.dram_tensor("qsn", [128, 194], BF16, kind="ExternalInput")
    yp = nc.dram_tensor("yp", [NQ, D], F32, kind="ExternalOutput")

    with TileContext(nc) as tc:
        with (
            tc.tile_pool(name="const", bufs=1) as constp,
            tc.tile_pool(name="persist", bufs=1) as pers,
            tc.tile_pool(name="vpool", bufs=NKT) as vpool,
        ):
            # ---- constants / tables ----
            onesb = constp.tile([128, 64], BF16, tag="onesb")
            nc.vector.memset(onesb[:], 1.0)
            eps_sb = constp.tile([128, 1], F32, tag="eps")
            nc.vector.memset(eps_sb[:], EPS)
            wq_sb = constp.tile([128, NDT * 256], BF16, tag="wq")
            c4q_sb = constp.tile([128, NQ], BF16, tag="c4q")
            s4q_sb = constp.tile([128, NQ], BF16, tag="s4q")
            perm_sb = constp.tile([128, 128], BF16, tag="perm")
            idw_sb = constp.tile([128, 128], BF16, tag="idw")
            qsn_sb = constp.tile([128, 194], BF16, tag="qsn")
            xq_sb = constp.tile([128, 2 * NDT * 512], BF16, tag="xq")
            wk_sb = constp.tile([128, NDT * 256], BF16, tag="wk")
            wv_sb = constp.tile([128, NDT * C], BF16, tag="wv")
            c4k_sb = constp.tile([128, T], BF16, tag="c4k")
            s4k_sb = constp.tile([128, T], BF16, tag="s4k")
            m01_sb = constp.tile([128, NKT * 128], BF16, tag="m01")
            wpn_sb = constp.tile([128, 2 * D], BF16, tag="wpn")

            # Q-path inputs first (sync ring); small consts on the
            # gpsimd ring so they don't head-of-line block the big ones
            nc.sync.dma_start(wq_sb[:], wq[:, :])
            for ch in range(2):
                nc.sync.dma_start(
                    xq_sb[:, 3072 * ch : 3072 * (ch + 1)],
                    xqp[:, 3072 * ch : 3072 * (ch + 1)],
                )
            nc.gpsimd.dma_start(perm_sb[:], perm[:, :])
            nc.gpsimd.dma_start(idw_sb[:], idw[:, :])
            nc.gpsimd.dma_start(qsn_sb[:], qsn[:, :])
            nc.gpsimd.dma_start(c4q_sb[:], c4q[:, :])
            nc.gpsimd.dma_start(s4q_sb[:], s4q[:, :])
            nc.sync.dma_start(wk_sb[:], wk[:, :])
            nc.sync.dma_start(wv_sb[:], wv[:, :])
            nc.gpsimd.dma_start(c4k_sb[:], c4k[:, :])
            nc.gpsimd.dma_start(s4k_sb[:], s4k[:, :])

            # ---- persistent q/k tiles (A: heads 0,1  B: head 2) ----
            qA = pers.tile([128, NQ], BF16, tag="qA")
            qB = pers.tile([128, NQ], BF16, tag="qB")
            kA = pers.tile([128, T], BF16, tag="kA")
            kB = pers.tile([128, T], BF16, tag="kB")
            rk_bf = pers.tile([97, T], BF16, tag="rkbf")
            rq_bf = pers.tile([97, NQ], BF16, tag="rqbf")

            # at: h0 rows 0-63 / h1 rows 64-127 (K-packed for yproj);
            # at2: h2 rows 0-63.  v blocks: h0 [hd@0:64, 1@64], h1
            # [1@0, hd@64:128], h2 [hd@0:64, 1@64] so pv rows align with at.
            at = pers.tile([128, NQ], BF16, tag="at")
            at2 = pers.tile([128, NQ], BF16, tag="at2")
            v_sb = []
            for t in range(NKT):
                vt = vpool.tile([128, 3 * 128], BF16, tag="v", name=f"v{t}")
                nc.vector.memset(vt[:], 0.0)
                v3m = vt[:].rearrange("p (b c) -> p b c", b=3, c=128)
                nc.vector.memset(v3m[:, 0::2, 64:65], 1.0)
                nc.vector.memset(vt[:, 128:129], 1.0)
                v_sb.append(vt)

            with (
                tc.tile_pool(name="xtp", bufs=2) as xtpool,
                tc.tile_pool(name="vinp", bufs=2) as vinpool,
                tc.tile_pool(name="sqp", bufs=2) as sqp,
                tc.tile_pool(name="ropep", bufs=4) as ropep,
                tc.tile_pool(name="rowp", bufs=2) as rowp,
            ):
                # ---- two-pass projections ----
                # pass1: raw A/B proj -> SBUF, squares (DVE), row-ssq matmuls
                # (rows 0=h0, 64=h2/B, 96=h1), one Ln + one Exp -> x^-0.5 rows.
                def proj_pass1(dstA, dstB, w_sb, x_sb, xoff, c0, rrow,
                               pool, emit, evac_act=True):
                    sq_a = sqp.tile([128, 512], BF16, tag="sq", name="sqa")
                    sq_b = sqp.tile([64, 512], BF16, tag="sq", name="sqb")

                    def one_proj(dst, coff, sqt):
                        ps = pool.tile([128, 512], F32, tag="pp", name="psp")
                        for d in range(NDT):
                            nc.tensor.matmul(
                                ps[:],
                                w_sb[:, 256 * d + coff : 256 * d + coff + 128],
                                x_sb[:, xoff + 512 * d : xoff + 512 * d + 512],
                                start=(d == 0),
                                stop=(d == NDT - 1),
                            )
                        nc.vector.tensor_copy(dst[:, c0 : c0 + 512], ps[:])
                        nc.vector.tensor_mul(
                            sqt[:],
                            dst[0 : sqt.shape[0], c0 : c0 + 512],
                            dst[0 : sqt.shape[0], c0 : c0 + 512],
                        )

                    def ssq_ln():
                        ssq = pool.tile([97, 512], F32, tag="pp", name="ssq")
                        nc.tensor.matmul(
                            ssq[:], qsn_sb[:, 0:97], sq_a[:],
                            start=True, stop=False, skip_group_check=True,
                        )
                        nc.tensor.matmul(
                            ssq[:], qsn_sb[0:64, 97:194], sq_b[0:64, :],
                            start=False, stop=True, skip_group_check=True,
                        )
                        ln1 = rowp.tile([97, 512], F32, tag="rqs", name="ln1")
                        nc.scalar.activation(
                            ln1[:], ssq[:], AF.Ln,
                            bias=eps_sb[0:97, :], scale=1.0 / HD,
                        )
                        nc.scalar.activation(
                            rrow[:, c0 : c0 + 512], ln1[:], AF.Exp, scale=-0.5
                        )

                    emit(lambda: one_proj(dstA, 0, sq_a))
                    emit(lambda: one_proj(dstB, 128, sq_b))
                    emit(ssq_ln)

                # pass2: rope via perm/idw PE combine; rms folded in the
                # final evacuation multiply.
                def rope_pass(tA, tB, rrow, c4t, s4t, c0, pool, emit,
                              evac_act=True):
                    rb = {}

                    def rbcopy(dst, srcp):
                        nc.vector.tensor_copy(dst, srcp)

                    def rb_a():
                        t = pool.tile([128, 512], F32, tag="pp", name="rba")
                        nc.tensor.matmul(
                            t[0:64, :], onesb[0:1, 0:64],
                            rrow[0:1, c0 : c0 + 512],
                            start=True, stop=True,
                            tile_position=(0, 0), skip_group_check=True,
                        )
                        nc.tensor.matmul(
                            t[64:128, :], onesb[96:97, 0:64],
                            rrow[96:97, c0 : c0 + 512],
                            start=True, stop=True,
                            tile_position=(96, 64), skip_group_check=True,
                        )
                        s = ropep.tile([128, 512], F32, tag="ropef", name="rbsb")
                        rbcopy(s[:], t[:])
                        rb["a"] = s

                    def rb_b():
                        t = pool.tile([128, 512], F32, tag="pp", name="rbb")
                        nc.tensor.matmul(
                            t[0:64, :], onesb[64:65, 0:64],
                            rrow[64:65, c0 : c0 + 512],
                            start=True, stop=True,
                            tile_position=(64, 0), skip_group_check=True,
                        )
                        nc.tensor.matmul(
                            t[64:128, :], onesb[64:65, 0:64],
                            rrow[64:65, c0 : c0 + 512],
                            start=True, stop=True,
                            tile_position=(64, 64), skip_group_check=True,
                        )
                        s = ropep.tile([128, 512], F32, tag="ropef", name="rbBsb")
                        rbcopy(s[:], t[:])
                        rb["b"] = s

                    def rot(tile_, key):
                        sl = tile_[:, c0 : c0 + 512]
                        m_ = ropep.tile([128, 512], BF16, tag="rope", name="m_")
                        e1 = ropep.tile([128, 512], BF16, tag="rope", name="e1")
                        nc.vector.tensor_mul(m_[:], sl, c4t[:, c0 : c0 + 512])
                        nc.vector.tensor_mul(e1[:], sl, s4t[:, c0 : c0 + 512])
                        qs = pool.tile([128, 512], F32, tag="pp", name="qs")
                        nc.tensor.matmul(
                            qs[:], perm_sb[:, :], e1[:],
                            start=True, stop=False, skip_group_check=True,
                        )
                        nc.tensor.matmul(
                            qs[:], idw_sb[:, :], m_[:],
                            start=False, stop=True, skip_group_check=True,
                        )
                        nc.vector.tensor_mul(sl, qs[:], rb[key][:])

                    emit(rb_a)
                    emit(rb_b)
                    emit(lambda: rot(tA, "a"))
                    emit(lambda: rot(tB, "b"))

                def kv_pass1(ch, pool, emit, evac_act=True):
                    c0 = 512 * ch
                    xt_ch = xtpool.tile([128, 3072], BF16, tag="xt", name="xt")
                    nc.sync.dma_start(
                        xt_ch[:], xtp[:, 3072 * ch : 3072 * (ch + 1)]
                    )
                    vi_t = vinpool.tile([128, 4 * C], BF16, tag="vin")
                    nc.sync.dma_start(
                        vi_t[:], vin[:, 4 * C * ch : 4 * C * (ch + 1)]
                    )
                    proj_pass1(kA, kB, wk_sb, xt_ch, 0, c0, rk_bf, pool,
                               emit, evac_act)

                    def one_v(ti):
                        t = 4 * ch + ti
                        ps = pool.tile([128, 512], F32, tag="pp", name="psv")
                        for d in range(NDT):
                            nc.tensor.matmul(
                                ps[:, 0:C],
                                xt_ch[:, 512 * d + 128 * ti : 512 * d + 128 * ti + 128],
                                wv_sb[:, C * d : C * (d + 1)],
                                start=(d == 0),
                                stop=(d == NDT - 1),
                            )
                        vt = v_sb[t]
                        v3 = vt[:].rearrange("p (b c) -> p b c", b=3, c=128)
                        p3 = ps[:, 0:C].rearrange("p (b c) -> p b c", b=3, c=64)
                        vi3 = vi_t[:, C * ti : C * (ti + 1)].rearrange(
                            "p (b c) -> p b c", b=3, c=64
                        )
                        nc.vector.tensor_add(
                            v3[:, 0::2, 0:64], p3[:, 0::2, :], vi3[:, 0::2, :]
                        )
                        nc.vector.tensor_add(
                            vt[:, 192:256],
                            ps[:, 64:128],
                            vi_t[:, C * ti + 64 : C * ti + 128],
                        )

                    for ti in range(4):
                        emit(lambda ti=ti: one_v(ti))

                # ---- phase A: Q + K/V chunks 0-1 + their ropes ----
                def run_now(f):
                    f()

                with tc.tile_pool(name="pA", bufs=4, space="PSUM") as pA:
                    for ch in range(2):
                        proj_pass1(qA, qB, wq_sb, xq_sb, 3072 * ch, 512 * ch,
                                   rq_bf, pA, run_now)
                    kv_pass1(0, pA, run_now)
                    kv_pass1(1, pA, run_now)
                    for ch in range(2):
                        rope_pass(qA, qB, rq_bf, c4q_sb, s4q_sb, 512 * ch,
                                  pA, run_now)
                    rope_pass(kA, kB, rk_bf, c4k_sb, s4k_sb, 0, pA, run_now)
                    rope_pass(kA, kB, rk_bf, c4k_sb, s4k_sb, 512, pA, run_now)

                nc.gpsimd.dma_start(m01_sb[:], m01[:, :])
                nc.gpsimd.dma_start(wpn_sb[:], wpn[:, :])

                # ---- phase B: loop1 (h0+h1) with chunks 2-3 proj pumped
                # into the ACT-bound stage gaps; then loop2 (h2, rowgroup
                # alternating via duplicated kB rows) ----
                with (
                    tc.tile_pool(name="etp", bufs=3) as etp,
                    tc.tile_pool(name="et2p", bufs=3) as et2p,
                    tc.tile_pool(name="rowd", bufs=4) as rowd,
                    tc.tile_pool(name="rbnp", bufs=3) as rbnp,
                    tc.tile_pool(name="ypool", bufs=3) as ypool,
                    tc.tile_pool(name="psP", bufs=2, space="PSUM") as psP,
                    tc.tile_pool(name="pvp", bufs=2, space="PSUM") as pvp,
                    tc.tile_pool(name="stq", bufs=2, space="PSUM") as stqp,
                ):
                    proj_q = []
                    pumped = [0]

                    def pump(n=1):
                        for _ in range(n):
                            if proj_q:
                                proj_q.pop(0)()
                                pumped[0] += 1

                    def pump_to(n):
                        while pumped[0] < n and proj_q:
                            proj_q.pop(0)()
                            pumped[0] += 1

                    emit_b = proj_q.append
                    kv_pass1(2, psP, emit_b, evac_act=False)
                    rope_pass(kA, kB, rk_bf, c4k_sb, s4k_sb, 1024, psP, emit_b,
                              evac_act=False)
                    kv_pass1(3, psP, emit_b, evac_act=False)
                    rope_pass(kA, kB, rk_bf, c4k_sb, s4k_sb, 1536, psP, emit_b,
                              evac_act=False)

                    pvs = {}
                    pvs2 = {}
                    st1 = {}
                    st2s = {}

                    def l1_flush(H):
                        s = st1.get(H)
                        if s is None:
                            return
                        k, sq_t, lo, w = s
                        j0 = j0_of(k)
                        et = etp.tile([128, 1024], BF16, tag="et", name=f"et{H}_{k}")
                        if w == 512:
                            nc.scalar.activation(et[:], sq_t[:], AF.Exp)
                        else:
                            nc.scalar.activation(et[:, 0:w], sq_t[:, 0:w], AF.Exp)
                            nc.scalar.activation(
                                et[:, 512 : 512 + w], sq_t[:, 512 : 512 + w],
                                AF.Exp,
                            )
                        if 4 * H <= j0 < 4 * (H + 1):
                            mk = m01_sb[:, 128 * k : 128 * (k + 1)]
                            nc.vector.tensor_mul(et[:, 0:128], et[:, 0:128], mk)
                            nc.vector.tensor_mul(
                                et[:, 512:640], et[:, 512:640], mk
                            )
                        first = k == 0
                        last = k == (7 if H == 0 else 15)
                        c0 = lo - 512 * H
                        pv0, pv1 = pvs[H]
                        vt = v_sb[k]
                        nc.tensor.matmul(
                            pv0[:, c0 : c0 + w], vt[:, 0:128], et[:, 0:w],
                            start=first, stop=last, skip_group_check=True,
                        )
                        nc.tensor.matmul(
                            pv1[:, c0 : c0 + w], vt[:, 128:256],
                            et[:, 512 : 512 + w],
                            start=first, stop=last, skip_group_check=True,
                        )
                        st1[H] = None

                    def l1_stage(H, k):
                        pump(1)
                        lo = max(512 * H, 128 * j0_of(k))
                        w = 512 * H + 512 - lo
                        if k == 0:
                            pvs[H] = [
                                pvp.tile([128, 512], F32, tag="pv",
                                         name=f"pv{H}{h}")
                                for h in range(2)
                            ]
                        sq_t = stqp.tile([128, 1024], F32, tag="stq",
                                         name=f"sq{H}_{k}")
                        ksl = slice(128 * k, 128 * (k + 1))
                        nc.tensor.matmul(
                            sq_t[:, 0:w], kA[0:64, ksl], qA[0:64, lo : lo + w],
                            start=True, stop=True, skip_group_check=True,
                        )
                        nc.tensor.matmul(
                            sq_t[:, 512 : 512 + w], kA[64:128, ksl],
                            qA[64:128, lo : lo + w],
                            start=True, stop=True, skip_group_check=True,
                        )
                        pump(1)
                        l1_flush(H)
                        st1[H] = (k, sq_t, lo, w)

                    def norm_head(den_row, pvsl, outsl, rb0, nm):
                        lnd = rowd.tile([1, 512], F32, tag="lnd", name=f"ln{nm}")
                        nc.scalar.activation(lnd[:], den_row, AF.Ln)
                        rden = rowd.tile([1, 512], BF16, tag="rden",
                                         name=f"rd{nm}")
                        nc.scalar.activation(rden[:], lnd[:], AF.Exp, scale=-1.0)
                        rbp = psP.tile([128, 512], F32, tag="pp", name=f"rp{nm}")
                        nc.tensor.matmul(
                            rbp[rb0 : rb0 + 64, 0:512], onesb[0:1, 0:64],
                            rden[:],
                            start=True, stop=True, skip_group_check=True,
                        )
                        rbn = rbnp.tile([128, 512], F32, tag="rbn",
                                        name=f"rb{nm}")
                        nc.vector.tensor_copy(
                            rbn[rb0 : rb0 + 64, :], rbp[rb0 : rb0 + 64, 0:512]
                        )
                        nc.vector.tensor_mul(outsl, pvsl, rbn[rb0 : rb0 + 64, :])

                    def norm1(H):
                        l1_flush(H)
                        pv0, pv1 = pvs[H]
                        cols = slice(512 * H, 512 * H + 512)
                        norm_head(pv0[64:65, :], pv0[0:64, :], at[0:64, cols],
                                  0, f"a{H}")
                        norm_head(pv1[0:1, :], pv1[64:128, :], at[64:128, cols],
                                  64, f"b{H}")

                    for k in range(8):
                        l1_stage(0, k)
                    l1_flush(0)
                    norm1(0)
                    for k in range(16):
                        if k == 8:
                            pump_to(11)  # chunk-2 rope must be applied
                        if k == 12:
                            pump_to(22)  # chunk-3 rope must be applied
                        l1_stage(1, k)
                    l1_flush(1)
                    norm1(1)
                    pump(len(proj_q))

                    # ---- loop2: h2, two key tiles per stage ----
                    def l2_flush(H):
                        s = st2s.get(H)
                        if s is None:
                            return
                        info, sq_t = s
                        w = info[0][2]
                        et = et2p.tile([128, 1024], BF16, tag="et2", name="e2")
                        nc.scalar.activation(
                            et[:, 0 : 512 + w], sq_t[:, 0 : 512 + w], AF.Exp
                        )
                        pv2 = pvs2[H]
                        for i, (k, lo, wk_) in enumerate(info):
                            j0 = j0_of(k)
                            if 4 * H <= j0 < 4 * (H + 1):
                                nc.vector.tensor_mul(
                                    et[:, 512 * i : 512 * i + 128],
                                    et[:, 512 * i : 512 * i + 128],
                                    m01_sb[:, 128 * k : 128 * (k + 1)],
                                )
                            nc.tensor.matmul(
                                pv2[:, lo - 512 * H : lo - 512 * H + wk_],
                                v_sb[k][:, 256:384],
                                et[:, 512 * i : 512 * i + wk_],
                                start=(k == 0),
                                stop=(k == (7 if H == 0 else 15)),
                                skip_group_check=True,
                            )
                        st2s[H] = None

                    def l2_stage(H, p, filler=None):
                        if p == 0:
                            pvs2[H] = pvp.tile([128, 512], F32, tag="pv",
                                               name=f"pv2{H}")
                        sq_t = stqp.tile([128, 1024], F32, tag="stq",
                                         name=f"s2{H}_{p}")
                        info = []
                        for i, k in enumerate((2 * p, 2 * p + 1)):
                            lo = max(512 * H, 128 * j0_of(k))
                            w = 512 * H + 512 - lo
                            rg = 64 * (k % 2)
                            nc.tensor.matmul(
                                sq_t[:, 512 * i : 512 * i + w],
                                kB[rg : rg + 64, 128 * k : 128 * (k + 1)],
                                qB[rg : rg + 64, lo : lo + w],
                                start=True, stop=True, skip_group_check=True,
                            )
                            info.append((k, lo, w))
                        if filler is not None:
                            filler()
                        l2_flush(H)
                        st2s[H] = (info, sq_t)

                    def norm2(H):
                        l2_flush(H)
                        pv2 = pvs2[H]
                        cols = slice(512 * H, 512 * H + 512)
                        norm_head(pv2[64:65, :], pv2[0:64, :], at2[0:64, cols],
                                  0, f"c{H}")

                    yts = {}

                    def yp_piece(j, n0, n1, evac_act):
                        ps = psP.tile([128, 512], F32, tag="pp",
                                      name=f"yp{j}_{n0}")
                        jsl = slice(128 * j, 128 * (j + 1))
                        nc.tensor.matmul(
                            ps[:, 0 : n1 - n0], at[:, jsl], wpn_sb[:, n0:n1],
                            start=True, stop=False, skip_group_check=True,
                        )
                        rg = 64 * (j % 2)
                        nc.tensor.matmul(
                            ps[:, 0 : n1 - n0], at2[rg : rg + 64, jsl],
                            wpn_sb[rg : rg + 64, D + n0 : D + n1],
                            start=False, stop=True, skip_group_check=True,
                        )
                        if j not in yts:
                            yts[j] = ypool.tile([128, D], F32, tag="y",
                                                name=f"yt{j}")
                        yt = yts[j]
                        if evac_act:
                            nc.scalar.activation(
                                yt[:, n0:n1], ps[:, 0 : n1 - n0], AF.Copy
                            )
                        else:
                            nc.vector.tensor_copy(yt[:, n0:n1], ps[:, 0 : n1 - n0])
                        if n1 == D:
                            nc.sync.dma_start(yp[jsl, :], yt[:])

                    for p in range(4):
                        l2_stage(0, p)
                    norm2(0)
                    nc.gpsimd.tensor_copy(at2[64:128, 0:512], at2[0:64, 0:512])
                    ypq = [(j, n0, n1) for j in range(4)
                           for n0, n1 in ((0, 512), (512, D))]

                    def yfill():
                        if ypq:
                            yp_piece(*ypq.pop(0), False)

                    for p in range(8):
                        l2_stage(1, p, filler=yfill)
                    l2_flush(1)
                    norm2(1)
                    nc.gpsimd.tensor_copy(
                        at2[64:128, 512:1024], at2[0:64, 512:1024]
                    )
                    while ypq:
                        yp_piece(*ypq.pop(0), False)
                    for j in range(4, 8):
                        for n0, n1 in ((0, 512), (512, D)):
                            yp_piece(j, n0, n1, n0 == 0)

    return nc


def _host_prep(x, vi, Wq, Wk, Wv, Wp, lamb):
    import ml_dtypes

    BF = ml_dtypes.bfloat16
    lam = float(lamb)
    xtf = np.ascontiguousarray(x[0].T, dtype=np.float32)  # [768, 2048]

    inv_freq = (1.0 / 10000.0) ** (np.arange(0, HD, 2, dtype=np.float32) / HD)
    tpos = np.arange(T, dtype=np.float32)
    freqs = np.outer(tpos, inv_freq).astype(np.float32)  # [T, 32]
    cosT = np.cos(freqs).T.astype(np.float32)  # [32, T]
    sinT = np.sin(freqs).T.astype(np.float32)
    c4 = np.vstack([cosT] * 4)  # [128, T]
    # swapped-sign sin stack: t_ = perm(raw * s4sw); perm swaps 0:32<->32:64
    #   rows 0:32 hold -sin (land on y2 = x2 c - x1 s)
    #   rows 32:64 hold +sin (land on y1 = x1 c + x2 s)
    s4sw = np.vstack([-sinT, sinT, -sinT, sinT])
    scale = float(1.0 / np.sqrt(np.float32(HD)))

    permf = np.zeros((128, 128), dtype=np.float32)
    for b in range(2):
        for i in range(32):
            permf[64 * b + 32 + i, 64 * b + i] = 1.0
            permf[64 * b + i, 64 * b + 32 + i] = 1.0
    idf = np.eye(128, dtype=np.float32)

    qsnf = np.zeros((128, 194), dtype=np.float32)
    qsnf[0:64, 0] = 1.0  # h0 ssq (sq_a rows 0-63) -> row 0
    qsnf[64:128, 96] = 1.0  # h1 ssq (sq_a rows 64-127) -> row 96
    qsnf[0:64, 97 + 64] = 1.0  # h2 ssq (sq_b rows 0-63) -> row 64

    tri01 = (np.arange(128)[None, :] >= np.arange(128)[:, None]).astype(
        np.float32
    )  # [p=tk, c=tq]: allowed iff c >= p
    vzf = np.zeros((128, 384), dtype=np.float32)
    vzf[:, 64] = 1.0   # h0 denominator column
    vzf[:, 128] = 1.0  # h1 denominator column
    vzf[:, 320] = 1.0  # h2 denominator column

    qcols_by_par = {}
    for par in (0, 1):
        jj = np.arange(8)
        qcols_by_par[par] = (
            256 * jj[:, None] + 128 * par + np.arange(128)[None, :]
        ).reshape(-1)

    def pack_chunks(mat, n_ch):
        # [768, n_ch*512] -> [128, n_ch*6*512] chunk-major, d-blocks inside
        cols = []
        for ch in range(n_ch):
            for d in range(NDT):
                cols.append(mat[128 * d : 128 * (d + 1), 512 * ch : 512 * (ch + 1)])
        return np.ascontiguousarray(np.concatenate(cols, axis=1))

    def pack_w(w):  # [768, 192] -> [128, 6*192]
        return np.ascontiguousarray(
            np.concatenate(
                [w[128 * d : 128 * (d + 1), :] for d in range(NDT)], axis=1
            )
        )

    def pack_w2(w):  # [768, 192] -> [128, 6*256]: per d [A(128)|h2 dup(2x64)]
        cols = []
        for d in range(NDT):
            blk = w[128 * d : 128 * (d + 1), :]
            cols.append(
                np.concatenate(
                    [blk[:, 0:128], blk[:, 128:192], blk[:, 128:192]], axis=1
                )
            )
        return np.ascontiguousarray(np.concatenate(cols, axis=1))

    in_maps = []
    for core in range(8):
        g, par = core // 2, core % 2
        cg = slice(C * g, C * (g + 1))
        qcols = qcols_by_par[par]
        mask = np.zeros((128, NKT * 128), dtype=np.float32)
        for k in range(NKT):
            gtile = 2 * j0_of(k) + par
            if gtile == k:
                mask[:, 128 * k : 128 * (k + 1)] = tri01
            elif gtile > k:
                mask[:, 128 * k : 128 * (k + 1)] = 1.0
        vinm = lam * vi[0][:, cg]  # [2048, 192]
        vinp = np.concatenate(
            [vinm[128 * t : 128 * (t + 1), :] for t in range(NKT)], axis=1
        )
        wpt = Wp[:, cg].T  # [192, 768]
        wpn = np.concatenate(
            [
                wpt[0:128, :],
                np.concatenate([wpt[128:192, :], wpt[128:192, :]], axis=0),
            ],
            axis=1,
        )  # [128, 1536]: pass1 rows h0;h1, pass2 rows h2 duplicated
        in_maps.append(
            {
                "xtp": pack_chunks(xtf, 4).astype(BF),
                "xqp": pack_chunks(
                    np.ascontiguousarray(xtf[:, qcols]), 2
                ).astype(BF),
                "wq": pack_w2(Wq[cg, :].T).astype(BF),
                "wk": pack_w2(Wk[cg, :].T).astype(BF),
                "wv": pack_w((1.0 - lam) * Wv[cg, :].T).astype(BF),
                "wpn": np.ascontiguousarray(wpn).astype(BF),
                "vin": np.ascontiguousarray(vinp).astype(BF),
                "vz": vzf.astype(BF),
                "c4k": c4.astype(BF),
                "s4k": s4sw.astype(BF),
                "c4q": np.ascontiguousarray(scale * c4[:, qcols]).astype(BF),
                "s4q": np.ascontiguousarray(scale * s4sw[:, qcols]).astype(BF),
                "m01": mask.astype(BF),
                "perm": permf.astype(BF),
                "idw": idf.astype(BF),
                "qsn": qsnf.astype(BF),
            }
        )
    return in_maps, qcols_by_par


def kernel(x, vi, Wq, Wk, Wv, Wp, lamb):
    from concourse.bass_utils import run_bass_kernel_spmd

    x = np.asarray(x, dtype=np.float32)
    vi = np.asarray(vi, dtype=np.float32)
    Wq = np.asarray(Wq, dtype=np.float32)
    Wk = np.asarray(Wk, dtype=np.float32)
    Wv = np.asarray(Wv, dtype=np.float32)
    Wp = np.asarray(Wp, dtype=np.float32)

    in_maps, qcols_by_par = _host_prep(x, vi, Wq, Wk, Wv, Wp, lamb)
    if "nc" not in _CACHED:
        _CACHED["nc"] = build_nc()
    nc = _CACHED["nc"]
    res = run_bass_kernel_spmd(
        nc, in_maps, core_ids=list(range(8)), trace=TRACE, tmpdir=TRACE_DIR
    )
    _CACHED["last_result"] = res

    y = np.zeros((T, D), dtype=np.float32)
    for core in range(8):
        y[qcols_by_par[core % 2]] += res.results[core]["yp"]
    return y[None]

